# revision 12
# baseline (speedup 1.0000x reference)
"""Trainium2 Bass kernel for nn_ModelNew_3556232922178 (dense_cnn).

Reference computation (B=16, Cin=32, D=H=W=32, Cout=64, k=3):
    y = ConvTranspose3d(x, W, stride=1, pad=0)      # full correlation, out 34^3
    y = (y + bias) * SCALE
    y = (y - running_mean) * rsqrt(running_var+EPS)  # inference BN
    out = y.mean over spatial                        # (B, Cout)

Because the global average pool sums over the *entire* full-correlation
output, every (input voxel, kernel tap) product contributes exactly once:
    sum_spatial(conv)[b,o] = sum_i (sum_spatial x)[b,i] * (sum_taps W)[o,i]
so the whole network collapses to a per-(b,i) spatial reduction of x, a
(B,Cin)x(Cin,Cout) matmul, and a per-channel affine:
    out[b,o] = M[b,o] * alpha[o] + beta[o]
    alpha[o] = SCALE/34^3 * rsqrt(rv[o]+EPS)
    beta[o]  = (bias[o]*SCALE - rm[o]) * rsqrt(rv[o]+EPS)

Sharding: data-parallel over batch, 2 batches per core, 8 cores. Each core
reduces its own x shard (8.4 MB — the dominant, DMA-bound cost), computes
its two output rows completely, no collectives. Host concatenates.

Device schedule per core (measured-trace-driven):
  x viewed as (2, 128, 8192): partition p = i*4 + q over (channel i,
  spatial quarter q) — a pure host reshape so every chunk DMA is a
  uniform-partition-stride 2-D AP. Batch 0 rides the SP HWDGE queue,
  batch 1 the ACT HWDGE queue; the two queues share all 16 DMA engines
  at ~420 GB/s aggregate (the per-core HBM ceiling), so the window is
  bytes-bound. 5 chunks per queue, descending (4096..256), so the final
  chunks — whose completion-semaphore propagation (~2 us) plus reduce
  sit on the critical path after the window closes — are small. The
  first 4 triggers per queue fire back-to-back at kernel entry; the 5th
  (the 9th/10th HWDGE DMA) carries Tile's proc-wrap wait on chunk 0's
  completion, which resolves mid-window before the queue drains, so it
  costs nothing. A dummy activation on the framework zero-constant runs
  while ACT's 5th trigger is blocked, pulling the 1.3 us activation
  table load off the post-wrap path. Batch 0 chunks reduce on DVE
  (reduce_sum), batch 1 on ACT (activation Copy + accum_out),
  queue==engine so every consumer carries exactly one sem wait (the
  walrus build rejects >1). Stats columns are row-summed into
  red[(i,q), b] (DVE), one K=128 PE matmul with the host-prepared
  tap-reduced replicated W^T folds the quarter-sum and channel
  contraction, and DVE applies the folded BN affine.

  The y store (512 B) is emitted OUTSIDE the Tile context, on the (by
  then idle) SP queue. SP program order places it after the exit drain
  chain, which already waits on DVE's final tick (the affine), so out_t
  is complete before the trigger with no extra semaphore. Nothing waits
  on the store's completion: the semaphore-file reset walls and the
  final barrier (~7 us of fixed NEFF epilogue) run while the store is
  in flight, and the store lands ~5 us before the final barrier
  retires. Correctness across re-executions is unaffected: the store
  carries no completion semaphore at all (verified in the BIR), and the
  next execution's first touch of out_t / y comes >30 us after its own
  start. Verified by the harness's 50-iteration re-execution check. ~15 us of the span is fixed NEFF
  preamble/epilogue (engine bring-up, register loads, semaphore reset
  walls) emitted by the framework lowering, present in any kernel.
"""

import numpy as np

import concourse.bass as bass
from concourse import mybir
from concourse.tile import TileContext
from concourse.vector_clock import ScopedClock
from concourse.bass_utils import run_bass_kernel_spmd

EPS = 1e-5
SCALE = 2.0
B, CIN, S = 16, 32, 32 * 32 * 32
COUT, KT = 64, 27
NCORES = 8
BPC = B // NCORES          # batches per core
Q = 4                      # spatial quarters -> 128 partitions
F = S // Q                 # 8192 elements per partition per batch
NSPATIAL = 34 * 34 * 34    # conv output positions (pool divisor)
# free-axis chunk sizes per batch. The first chunk is SMALL: a 2 MB lead
# chunk's 128 descriptors monopolize all 16 DMA engines for ~3 us, which
# delays the OTHER queue's start by that much (measured); a 512-elem lead
# drains in ~0.6 us so both queues stream within ~1 us of each other. The
# middle descends so the final chunks — whose completion-semaphore
# propagation (~2 us) plus reduce sit on the critical path after the
# window closes — are small.
CHUNKS = [512, 4096, 2048, 1152, 384]
assert sum(CHUNKS) == F
F32 = mybir.dt.float32

# If True, the y store is emitted outside the Tile context and nothing
# waits on its DMA completion (the ~7 us fixed epilogue provides the
# slack it needs to land). If False, the store is Tile-tracked and the
# exit drain waits for it (~2.5 us slower, maximally conservative).
UNTRACKED_STORE = True

TRACE = False              # set by test harness to collect an NTFF profile
LAST_RESULT = None         # BassKernelResults of the most recent run


class SplitDrainTileContext(TileContext):
    """TileContext whose exit drain splits sem waits across multiple drains.

    The walrus build here rejects any instruction carrying more than one
    sync wait ("Too many sync wait commands"). Tile's stock exit path puts
    every outstanding proc's wait on a single drain, so any kernel touching
    2+ logical processors fails codegen. Sequential single-wait drains on
    the same engine are semantically identical.
    """

    def _drain_and_barrier(self, tick_clock, wait_clock):
        drain_inst = self.nc.sync.drain()
        wait_clock.add_sem_waits(
            drain_inst.ins, ScopedClock({None: tick_clock.global_clock})
        )
        si = drain_inst.ins.sync_info
        waits = list(si.on_wait) if si is not None and si.on_wait else []
        updates = list(si.on_update) if si is not None and si.on_update else []
        # Poll order matters: each split drain polls its sem sequentially
        # (~0.2 us/poll). Sort so the latest completions are polled last —
        # everything else has long completed by then, so no polls remain
        # after the last producer lands.
        waits.sort(key=lambda w: (w.wait_value, w.ant_name or ""))
        last_drain = drain_inst
        if len(waits) > 1:
            drain_inst.ins.sync_info = mybir.SyncInfo(on_wait=waits[:1], on_update=[])
            for i, w in enumerate(waits[1:]):
                extra = self.nc.sync.drain()
                is_last = i == len(waits) - 2
                extra.ins.sync_info = mybir.SyncInfo(
                    on_wait=[w], on_update=updates if is_last else []
                )
                last_drain = extra

        # Stock Tile brackets the sem reset with two all-engine barriers
        # (leader/follower drains, ~4 us each). The split drains above
        # already wait on every proc's final tick, so a single sem gate
        # (SP drain -> gpsimd clear) gives the same ordering for free.
        # Re-executability is verified by the test harness.
        gate = self.nc.alloc_semaphore("tile_exit_gate")
        last_drain.then_inc(gate, 1)
        self.nc.gpsimd.wait_ge(gate, 1)
        assert self.sems is not None
        popped = self.nc._tile_sem_poison_stack.pop()
        assert popped is self._sem_poison
        self.nc.clear_and_free_semaphores(
            list(self.sems.allocated().values()) + [gate]
        )


def _build_program():
    nc = bass.Bass()
    x = nc.dram_tensor("x", (BPC, 128, F), F32, kind="ExternalInput")
    # Host-prepared tap-reduced W^T replicated over the 4 quarter groups
    # (static-weight preprocessing, same as BN/conv folding):
    # w[(i*4+q), o] = sum_t weight[o, i, t]
    w = nc.dram_tensor("w", (128, COUT), F32, kind="ExternalInput")
    # Host-folded BN affine constants (inference BN folding):
    # ab[:, 0] = SCALE/34^3 * rsqrt(rv+EPS), ab[:, 1] = (bias*SCALE-rm)*rsqrt(rv+EPS)
    ab = nc.dram_tensor("ab", (COUT, 2), F32, kind="ExternalInput")
    y = nc.dram_tensor("y", (COUT, BPC), F32, kind="ExternalOutput")

    # Completion sem for the untracked y store (walrus rejects a DGE with
    # no sync info). Allocated FIRST so it gets the lowest kernel sem
    # number deterministically — nothing ever waits on it, and the NEFF
    # epilogue's semaphore reset wall zeroes it each run.
    y_sem = nc.alloc_semaphore("y_store_sem") if UNTRACKED_STORE else None

    with SplitDrainTileContext(nc) as tc:
        with (
            tc.tile_pool(name="const", bufs=1) as const,
            # per-chunk tags: each chunk gets its own exact-size slot, so
            # chunk DMAs carry no WAR/WAW waits (each instruction may carry
            # at most ONE wait) and SBUF stays within budget
            tc.tile_pool(name="xbuf", bufs=1) as xbuf,
            tc.tile_pool(name="ps", bufs=1, space="PSUM") as ps,
        ):
            # x chunk triggers, interleaved SP/ACT. The first 4 per queue
            # have no waits and fire back-to-back at kernel entry; the 5th
            # pair picks up Tile's proc-wrap wait (chunk 0 completion),
            # which resolves mid-window before its queue drains.
            xts = {}
            for k, sz in enumerate(CHUNKS):
                start = sum(CHUNKS[:k])
                for b, eng in ((0, nc.sync), (1, nc.scalar)):
                    xt = xbuf.tile([128, sz], F32, name=f"x{b}_{k}", tag=f"x{b}_{k}")
                    eng.dma_start(out=xt, in_=x[b, :, start : start + sz])
                    xts[(b, k)] = xt

            # Tap-reduced replicated W^T (128, 64) + affine consts — tiny,
            # via SWDGE (q0), overlapped with the x window.
            wsum = const.tile([128, COUT], F32)
            nc.gpsimd.dma_start(out=wsum, in_=w[:, :])
            ab_t = const.tile([COUT, 2], F32)
            nc.gpsimd.dma_start(out=ab_t, in_=ab[:, :])

            # DVE-side copies of the small SWDGE inputs so matmul/affine
            # operands are DVE-produced and carry a single sem wait. DVE is
            # idle until the first big chunk lands, so these are free.
            wsum_s = const.tile([128, COUT], F32)
            ab_s = const.tile([COUT, 2], F32)
            nc.vector.tensor_copy(wsum_s, wsum)
            nc.vector.tensor_copy(ab_s, ab_t)

            # Dummy activation on the framework zero-constant: ready
            # immediately, so the Tile scheduler runs it while ACT's 5th
            # trigger is blocked on its proc-wrap wait — pulling the
            # ~1.3 us activation-table load off the critical path.
            warm = const.tile([128, 1], F32)
            nc.scalar.activation(
                out=warm,
                in_=nc.const_aps.aps[(F32, 0.0)],
                func=mybir.ActivationFunctionType.Copy,
            )

            # Spatial reduction: batch 0 chunks on DVE, batch 1 on ACT
            # (activation Copy + accum_out), one stats column per chunk.
            nch = len(CHUNKS)
            stats_d = const.tile([128, nch], F32)
            stats_a = const.tile([128, nch], F32)
            for k, sz in enumerate(CHUNKS):
                nc.vector.reduce_sum(
                    out=stats_d[:, k : k + 1],
                    in_=xts[(0, k)],
                    axis=mybir.AxisListType.X,
                )
            for k, sz in enumerate(CHUNKS):
                scratch = const.tile(
                    [128, sz], F32, name=f"scr{k}", tag=f"scr{k}"
                )
                nc.scalar.activation(
                    out=scratch,
                    in_=xts[(1, k)],
                    func=mybir.ActivationFunctionType.Copy,
                    accum_out=stats_a[:, k : k + 1],
                )

            # Per-batch chunk sums: red[(i,q), b]
            red = const.tile([128, BPC], F32)
            nc.vector.reduce_sum(
                out=red[:, 0:1], in_=stats_d[:, :], axis=mybir.AxisListType.X
            )
            nc.vector.reduce_sum(
                out=red[:, 1:2], in_=stats_a[:, :], axis=mybir.AxisListType.X
            )

            # psum[o, b] = sum_{(q,i)} wsum[(q,i), o] * red[(q,i), b]:
            # one K=128 matmul folds the quarter-sum and channel contraction.
            pm = ps.tile([COUT, BPC], F32)
            nc.tensor.matmul(pm, wsum_s, red, start=True, stop=True)

            out_t = const.tile([COUT, BPC], F32)
            nc.vector.tensor_scalar(                            # waits PE only
                out=out_t,
                in0=pm,
                scalar1=ab_s[:, 0:1],
                scalar2=ab_s[:, 1:2],
                op0=mybir.AluOpType.mult,
                op1=mybir.AluOpType.add,
            )
            if not UNTRACKED_STORE:
                nc.sync.dma_start(out=y[:, :], in_=out_t)

    if UNTRACKED_STORE:
        # Raw (un-Tile-tracked) y store on the long-idle SP queue. SP
        # program order places it after the exit drain chain, which waits
        # on DVE's final tick (the affine) — so out_t is complete before
        # the trigger, with no extra semaphore. Nothing waits on the
        # store's completion: it lands during the fixed ~7 us NEFF
        # epilogue (semaphore reset walls + final barrier).
        store = nc.sync.dma_start(out=y[:, :], in_=out_t).then_inc(y_sem, 16)
        _hoist_y_store(nc, store.ins)

    _elide_y_store_wrap_wait(nc)
    return nc


def _hoist_y_store(nc, store_inst):
    """Move the raw y store up SP's exit-drain chain.

    The store's only data dependency is out_t (DVE's final tick). The split
    drain chain polls every proc's sem one drain at a time; the store sits
    after ALL of them, but only the DVE-tick drain orders its dependency.
    Hoist it to immediately after the last DVE-waiting drain so the store
    triggers ~0.5 us earlier; the remaining drains (chunk DMA sems, other
    engine ticks — all long satisfied by then) poll afterwards.
    """
    for f in nc.m.functions:
        for bb in f.blocks:
            insts = bb.instructions
            if store_inst not in insts:
                continue
            dve_idx = None
            for i, inst in enumerate(insts):
                if inst is store_inst:
                    break
                if type(inst).__name__ == "InstDrain" and inst.sync_info:
                    waits = inst.sync_info.on_wait or []
                    if any((w.ant_name or "").startswith("DVE") for w in waits):
                        dve_idx = i
            assert dve_idx is not None, "no DVE-waiting drain found before store"
            insts.remove(store_inst)
            insts.insert(dve_idx + 1, store_inst)
            return
    raise AssertionError("y store instruction not found in any block")


def _elide_y_store_wrap_wait(nc):
    """Drop the DMAHW proc-wrap wait from any instruction that also has a
    DVE data wait (the tracked y store). The wrap wait's proc carries an
    earlier x chunk whose reduce -> red -> matmul -> affine chain precedes
    out_t, so the wait is transitively implied and safe to elide (the
    codegen rejects instructions with more than one sem wait). A no-op for
    the untracked-store build, where no instruction carries two waits.
    """
    stripped = 0
    for f in nc.m.functions:
        for bb in f.blocks:
            for inst in bb.instructions:
                si = inst.sync_info
                if si is None or not si.on_wait or len(si.on_wait) < 2:
                    continue
                names = [w.ant_name or "" for w in si.on_wait]
                keep = [
                    w for w in si.on_wait if not (w.ant_name or "").startswith("DMAHW")
                ]
                assert len(keep) == 1 and keep[0].ant_name.startswith("DVE"), names
                inst.sync_info = mybir.SyncInfo(
                    on_wait=keep, on_update=list(si.on_update or [])
                )
                stripped += 1
    assert stripped <= 1, f"expected at most the y store, stripped {stripped}"


def prep_inputs(x, weight, bias, running_mean, running_var):
    """Host-side sharding prep: per-core in_maps for run_bass_kernel_spmd."""
    x = np.ascontiguousarray(np.asarray(x, dtype=np.float32))
    weight = np.ascontiguousarray(np.asarray(weight, dtype=np.float32))
    bias = np.ascontiguousarray(np.asarray(bias, dtype=np.float32))
    rm = np.ascontiguousarray(np.asarray(running_mean, dtype=np.float32))
    rv = np.ascontiguousarray(np.asarray(running_var, dtype=np.float32))

    xv = x.reshape(B, 128, F)          # (b, i*4+q, f) — contiguous view
    # Static weight preprocessing (BN/conv-fold style): tap-reduce W and
    # replicate W^T across the 4 quarter groups, i-outer to match x (32 KB)
    wv = np.ascontiguousarray(
        np.repeat(
            weight.reshape(COUT, CIN, KT).sum(axis=2).T.astype(np.float32), Q, axis=0
        )
    )
    rstd = (1.0 / np.sqrt(rv + np.float32(EPS))).astype(np.float32)
    alpha = (np.float32(SCALE / NSPATIAL) * rstd).astype(np.float32)
    beta = ((bias * np.float32(SCALE) - rm) * rstd).astype(np.float32)
    ab = np.ascontiguousarray(np.stack([alpha, beta], axis=1))
    return [
        {"x": xv[k * BPC : (k + 1) * BPC], "w": wv, "ab": ab}
        for k in range(NCORES)
    ]


def kernel(x, weight, bias, running_mean, running_var):
    global LAST_RESULT
    in_maps = prep_inputs(x, weight, bias, running_mean, running_var)
    nc = _build_program()
    res = run_bass_kernel_spmd(
        nc, in_maps, core_ids=list(range(NCORES)), trace=TRACE
    )
    LAST_RESULT = res

    out = np.empty((B, COUT), dtype=np.float32)
    for k in range(NCORES):
        out[k * BPC : (k + 1) * BPC] = res.results[k]["y"].T
    return out


# revision 15
# speedup vs baseline: 1.0931x; 1.0931x over previous
"""Trainium2 Bass kernel for nn_ModelNew_3556232922178 (dense_cnn).

Reference computation (B=16, Cin=32, D=H=W=32, Cout=64, k=3):
    y = ConvTranspose3d(x, W, stride=1, pad=0)      # full correlation, out 34^3
    y = (y + bias) * SCALE
    y = (y - running_mean) * rsqrt(running_var+EPS)  # inference BN
    out = y.mean over spatial                        # (B, Cout)

Because the global average pool sums over the *entire* full-correlation
output, every (input voxel, kernel tap) product contributes exactly once:
    sum_spatial(conv)[b,o] = sum_i (sum_spatial x)[b,i] * (sum_taps W)[o,i]
so the whole network collapses to a per-(b,i) spatial reduction of x, a
(B,Cin)x(Cin,Cout) matmul, and a per-channel affine:
    out[b,o] = M[b,o] * alpha[o] + beta[o]
    alpha[o] = SCALE/34^3 * rsqrt(rv[o]+EPS)
    beta[o]  = (bias[o]*SCALE - rm[o]) * rsqrt(rv[o]+EPS)

Sharding: data-parallel over batch, 2 batches per core, 8 cores. Each core
reduces its own x shard (8.4 MB — the dominant, DMA-bound cost), computes
its two output rows completely, no collectives. Host concatenates.

Device schedule per core (measured-trace-driven):
  x viewed as (2, 128, 8192): partition p = i*4 + q over (channel i,
  spatial quarter q) — a pure host reshape so every chunk DMA is a
  uniform-partition-stride 2-D AP. Batch 0 rides the SP HWDGE queue,
  batch 1 the ACT HWDGE queue; the two queues share all 16 DMA engines
  at ~420 GB/s aggregate (the per-core HBM ceiling), so the window is
  bytes-bound. 5 chunks per queue, descending (4096..256), so the final
  chunks — whose completion-semaphore propagation (~2 us) plus reduce
  sit on the critical path after the window closes — are small. The
  first 4 triggers per queue fire back-to-back at kernel entry; the 5th
  (the 9th/10th HWDGE DMA) carries Tile's proc-wrap wait on chunk 0's
  completion, which resolves mid-window before the queue drains, so it
  costs nothing. A dummy activation on the framework zero-constant runs
  while ACT's 5th trigger is blocked, pulling the 1.3 us activation
  table load off the post-wrap path. Batch 0 chunks reduce on DVE
  (reduce_sum), batch 1 on ACT (activation Copy + accum_out),
  queue==engine so every consumer carries exactly one sem wait (the
  walrus build rejects >1). Stats columns are row-summed into
  red[(i,q), b] (DVE), one K=128 PE matmul with the host-prepared
  tap-reduced replicated W^T folds the quarter-sum and channel
  contraction, and DVE applies the folded BN affine.

  The y store (512 B) is emitted OUTSIDE the Tile context, on the (by
  then idle) SP queue. SP program order places it after the exit drain
  chain, which already waits on DVE's final tick (the affine), so out_t
  is complete before the trigger with no extra semaphore. Nothing waits
  on the store's completion: the semaphore-file reset walls and the
  final barrier (~7 us of fixed NEFF epilogue) run while the store is
  in flight, and the store lands ~5 us before the final barrier
  retires. Correctness across re-executions is unaffected: the store
  carries no completion semaphore at all (verified in the BIR), and the
  next execution's first touch of out_t / y comes >30 us after its own
  start. Verified by the harness's 50-iteration re-execution check. ~15 us of the span is fixed NEFF
  preamble/epilogue (engine bring-up, register loads, semaphore reset
  walls) emitted by the framework lowering, present in any kernel.
"""

import numpy as np

import concourse.bass as bass
from concourse import mybir
from concourse.tile import TileContext
from concourse.vector_clock import ScopedClock
from concourse.bass_utils import run_bass_kernel_spmd

EPS = 1e-5
SCALE = 2.0
B, CIN, S = 16, 32, 32 * 32 * 32
COUT, KT = 64, 27
NCORES = 8
BPC = B // NCORES          # batches per core
Q = 4                      # spatial quarters -> 128 partitions
F = S // Q                 # 8192 elements per partition per batch
NSPATIAL = 34 * 34 * 34    # conv output positions (pool divisor)
# free-axis chunk sizes per batch. Descriptors are one partition-row of a
# chunk (size*4 bytes): 8-16 KB descriptors sustain the full ~420 GB/s,
# but <=2 KB descriptors collapse to ~40 GB/s (measured), so no chunk may
# be small. The last chunk is the smallest viable (6 KB descriptors) since
# its completion-semaphore propagation (~2 us) plus reduce sit on the
# critical path after the window closes.
CHUNKS = [2560, 2048, 2048, 1536]
assert sum(CHUNKS) == F
F32 = mybir.dt.float32

# If True, the y store is emitted outside the Tile context and nothing
# waits on its DMA completion (the ~7 us fixed epilogue provides the
# slack it needs to land). If False, the store is Tile-tracked and the
# exit drain waits for it (~2.5 us slower, maximally conservative).
UNTRACKED_STORE = True

TRACE = False              # set by test harness to collect an NTFF profile
LAST_RESULT = None         # BassKernelResults of the most recent run


class SplitDrainTileContext(TileContext):
    """TileContext whose exit drain splits sem waits across multiple drains.

    The walrus build here rejects any instruction carrying more than one
    sync wait ("Too many sync wait commands"). Tile's stock exit path puts
    every outstanding proc's wait on a single drain, so any kernel touching
    2+ logical processors fails codegen. Sequential single-wait drains on
    the same engine are semantically identical.
    """

    def _drain_and_barrier(self, tick_clock, wait_clock):
        drain_inst = self.nc.sync.drain()
        wait_clock.add_sem_waits(
            drain_inst.ins, ScopedClock({None: tick_clock.global_clock})
        )
        si = drain_inst.ins.sync_info
        waits = list(si.on_wait) if si is not None and si.on_wait else []
        updates = list(si.on_update) if si is not None and si.on_update else []
        # Prune transitively-implied waits: every chunk-DMA sem (DMAHW*) is
        # consumed by its reduce before that engine's tick; the SWDGE const
        # sems (DMASW*) by the DVE copies; the PE tick by the affine; the
        # ACT tick by red_a (a DVE op). All therefore precede DVE's final
        # tick (the affine), so a single DVE-tick drain orders everything.
        # Anything unrecognized is conservatively kept.
        implied = ("DMAHW", "DMASW", "PE_", "Activation_")
        kept = [w for w in waits if not (w.ant_name or "").startswith(implied)]
        assert any((w.ant_name or "").startswith("DVE") for w in kept), [
            w.ant_name for w in waits
        ]
        waits = kept
        # Poll order matters: each split drain polls its sem sequentially
        # (~0.2 us/poll). Sort so the latest completions are polled last —
        # everything else has long completed by then, so no polls remain
        # after the last producer lands.
        waits.sort(key=lambda w: (w.wait_value, w.ant_name or ""))
        if len(waits) == 1:
            drain_inst.ins.sync_info = mybir.SyncInfo(
                on_wait=waits, on_update=updates
            )
        last_drain = drain_inst
        if len(waits) > 1:
            drain_inst.ins.sync_info = mybir.SyncInfo(on_wait=waits[:1], on_update=[])
            for i, w in enumerate(waits[1:]):
                extra = self.nc.sync.drain()
                is_last = i == len(waits) - 2
                extra.ins.sync_info = mybir.SyncInfo(
                    on_wait=[w], on_update=updates if is_last else []
                )
                last_drain = extra

        # Stock Tile brackets the sem reset with two all-engine barriers
        # (leader/follower drains, ~4 us each). The split drains above
        # already wait on every proc's final tick, so a single sem gate
        # (SP drain -> gpsimd clear) gives the same ordering for free.
        # Re-executability is verified by the test harness.
        gate = self.nc.alloc_semaphore("tile_exit_gate")
        last_drain.then_inc(gate, 1)
        self.nc.gpsimd.wait_ge(gate, 1)
        assert self.sems is not None
        popped = self.nc._tile_sem_poison_stack.pop()
        assert popped is self._sem_poison
        self.nc.clear_and_free_semaphores(
            list(self.sems.allocated().values()) + [gate]
        )


def _build_program():
    nc = bass.Bass()
    x = nc.dram_tensor("x", (BPC, 128, F), F32, kind="ExternalInput")
    # Host-prepared tap-reduced W^T replicated over the 4 quarter groups
    # (static-weight preprocessing, same as BN/conv folding):
    # w[(i*4+q), o] = sum_t weight[o, i, t]
    w = nc.dram_tensor("w", (128, COUT), F32, kind="ExternalInput")
    # Host-folded BN affine constants (inference BN folding):
    # ab[:, 0] = SCALE/34^3 * rsqrt(rv+EPS), ab[:, 1] = (bias*SCALE-rm)*rsqrt(rv+EPS)
    ab = nc.dram_tensor("ab", (COUT, 2), F32, kind="ExternalInput")
    y = nc.dram_tensor("y", (COUT, BPC), F32, kind="ExternalOutput")

    # Completion sem for the untracked y store (walrus rejects a DGE with
    # no sync info). Allocated FIRST so it gets the lowest kernel sem
    # number deterministically — nothing ever waits on it, and the NEFF
    # epilogue's semaphore reset wall zeroes it each run.
    y_sem = nc.alloc_semaphore("y_store_sem") if UNTRACKED_STORE else None

    with SplitDrainTileContext(nc) as tc:
        with (
            tc.tile_pool(name="const", bufs=1) as const,
            # per-chunk tags: each chunk gets its own exact-size slot, so
            # chunk DMAs carry no WAR/WAW waits (each instruction may carry
            # at most ONE wait) and SBUF stays within budget
            tc.tile_pool(name="xbuf", bufs=1) as xbuf,
            tc.tile_pool(name="ps", bufs=1, space="PSUM") as ps,
        ):
            # x chunk triggers, interleaved SP/ACT. The first 4 per queue
            # have no waits and fire back-to-back at kernel entry; the 5th
            # pair picks up Tile's proc-wrap wait (chunk 0 completion),
            # which resolves mid-window before its queue drains.
            # ACT's queue (batch 1) triggers first: the first-triggered
            # queue's lead chunk monopolizes the DMA engines for ~2.5 us, so
            # the first queue's chunks arrive that much earlier — give the
            # head start to ACT, which carries extra serial overhead (the
            # activation-table load and per-chunk accumulator reads).
            xts = {}
            for k, sz in enumerate(CHUNKS):
                start = sum(CHUNKS[:k])
                for b, eng in ((1, nc.scalar), (0, nc.sync)):
                    xt = xbuf.tile([128, sz], F32, name=f"x{b}_{k}", tag=f"x{b}_{k}")
                    eng.dma_start(out=xt, in_=x[b, :, start : start + sz])
                    xts[(b, k)] = xt

            # Tap-reduced replicated W^T (128, 64) + affine consts — tiny,
            # via SWDGE (q0), overlapped with the x window.
            wsum = const.tile([128, COUT], F32)
            nc.gpsimd.dma_start(out=wsum, in_=w[:, :])
            ab_t = const.tile([COUT, 2], F32)
            nc.gpsimd.dma_start(out=ab_t, in_=ab[:, :])

            # DVE-side copies of the small SWDGE inputs so matmul/affine
            # operands are DVE-produced and carry a single sem wait. DVE is
            # idle until the first big chunk lands, so these are free.
            wsum_s = const.tile([128, COUT], F32)
            ab_s = const.tile([COUT, 2], F32)
            nc.vector.tensor_copy(wsum_s, wsum)
            nc.vector.tensor_copy(ab_s, ab_t)

            # Dummy activation on the framework zero-constant: ready
            # immediately, so the Tile scheduler runs it while ACT's 5th
            # trigger is blocked on its proc-wrap wait — pulling the
            # ~1.3 us activation-table load off the critical path.
            warm = const.tile([128, 1], F32)
            nc.scalar.activation(
                out=warm,
                in_=nc.const_aps.aps[(F32, 0.0)],
                func=mybir.ActivationFunctionType.Copy,
            )

            # Spatial reduction: batch 0 chunks on DVE, batch 1 on ACT
            # (activation Copy + accum_out), one stats column per chunk.
            nch = len(CHUNKS)
            stats_d = const.tile([128, nch], F32)
            stats_a = const.tile([128, nch], F32)
            for k, sz in enumerate(CHUNKS):
                nc.vector.reduce_sum(
                    out=stats_d[:, k : k + 1],
                    in_=xts[(0, k)],
                    axis=mybir.AxisListType.X,
                )
            for k, sz in enumerate(CHUNKS):
                scratch = const.tile(
                    [128, sz], F32, name=f"scr{k}", tag=f"scr{k}"
                )
                nc.scalar.activation(
                    out=scratch,
                    in_=xts[(1, k)],
                    func=mybir.ActivationFunctionType.Copy,
                    accum_out=stats_a[:, k : k + 1],
                )

            # Per-batch chunk sums: red[(i,q), b]
            red = const.tile([128, BPC], F32)
            nc.vector.reduce_sum(
                out=red[:, 0:1], in_=stats_d[:, :], axis=mybir.AxisListType.X
            )
            nc.vector.reduce_sum(
                out=red[:, 1:2], in_=stats_a[:, :], axis=mybir.AxisListType.X
            )

            # psum[o, b] = sum_{(q,i)} wsum[(q,i), o] * red[(q,i), b]:
            # one K=128 matmul folds the quarter-sum and channel contraction.
            pm = ps.tile([COUT, BPC], F32)
            nc.tensor.matmul(pm, wsum_s, red, start=True, stop=True)

            out_t = const.tile([COUT, BPC], F32)
            nc.vector.tensor_scalar(                            # waits PE only
                out=out_t,
                in0=pm,
                scalar1=ab_s[:, 0:1],
                scalar2=ab_s[:, 1:2],
                op0=mybir.AluOpType.mult,
                op1=mybir.AluOpType.add,
            )
            if not UNTRACKED_STORE:
                nc.sync.dma_start(out=y[:, :], in_=out_t)

    if UNTRACKED_STORE:
        # Raw (un-Tile-tracked) y store on the long-idle SP queue. SP
        # program order places it after the exit drain chain, which waits
        # on DVE's final tick (the affine) — so out_t is complete before
        # the trigger, with no extra semaphore. Nothing waits on the
        # store's completion: it lands during the fixed ~7 us NEFF
        # epilogue (semaphore reset walls + final barrier).
        store = nc.sync.dma_start(out=y[:, :], in_=out_t).then_inc(y_sem, 16)
        _hoist_y_store(nc, store.ins)

    _elide_y_store_wrap_wait(nc)
    return nc


def _hoist_y_store(nc, store_inst):
    """Move the raw y store up SP's exit-drain chain.

    The store's only data dependency is out_t (DVE's final tick). The split
    drain chain polls every proc's sem one drain at a time; the store sits
    after ALL of them, but only the DVE-tick drain orders its dependency.
    Hoist it to immediately after the last DVE-waiting drain so the store
    triggers ~0.5 us earlier; the remaining drains (chunk DMA sems, other
    engine ticks — all long satisfied by then) poll afterwards.
    """
    for f in nc.m.functions:
        for bb in f.blocks:
            insts = bb.instructions
            if store_inst not in insts:
                continue
            dve_idx = None
            for i, inst in enumerate(insts):
                if inst is store_inst:
                    break
                if type(inst).__name__ == "InstDrain" and inst.sync_info:
                    waits = inst.sync_info.on_wait or []
                    if any((w.ant_name or "").startswith("DVE") for w in waits):
                        dve_idx = i
            assert dve_idx is not None, "no DVE-waiting drain found before store"
            insts.remove(store_inst)
            insts.insert(dve_idx + 1, store_inst)
            return
    raise AssertionError("y store instruction not found in any block")


def _elide_y_store_wrap_wait(nc):
    """Drop the DMAHW proc-wrap wait from any instruction that also has a
    DVE data wait (the tracked y store). The wrap wait's proc carries an
    earlier x chunk whose reduce -> red -> matmul -> affine chain precedes
    out_t, so the wait is transitively implied and safe to elide (the
    codegen rejects instructions with more than one sem wait). A no-op for
    the untracked-store build, where no instruction carries two waits.
    """
    stripped = 0
    for f in nc.m.functions:
        for bb in f.blocks:
            for inst in bb.instructions:
                si = inst.sync_info
                if si is None or not si.on_wait or len(si.on_wait) < 2:
                    continue
                names = [w.ant_name or "" for w in si.on_wait]
                keep = [
                    w for w in si.on_wait if not (w.ant_name or "").startswith("DMAHW")
                ]
                assert len(keep) == 1 and keep[0].ant_name.startswith("DVE"), names
                inst.sync_info = mybir.SyncInfo(
                    on_wait=keep, on_update=list(si.on_update or [])
                )
                stripped += 1
    assert stripped <= 1, f"expected at most the y store, stripped {stripped}"


def prep_inputs(x, weight, bias, running_mean, running_var):
    """Host-side sharding prep: per-core in_maps for run_bass_kernel_spmd."""
    x = np.ascontiguousarray(np.asarray(x, dtype=np.float32))
    weight = np.ascontiguousarray(np.asarray(weight, dtype=np.float32))
    bias = np.ascontiguousarray(np.asarray(bias, dtype=np.float32))
    rm = np.ascontiguousarray(np.asarray(running_mean, dtype=np.float32))
    rv = np.ascontiguousarray(np.asarray(running_var, dtype=np.float32))

    xv = x.reshape(B, 128, F)          # (b, i*4+q, f) — contiguous view
    # Static weight preprocessing (BN/conv-fold style): tap-reduce W and
    # replicate W^T across the 4 quarter groups, i-outer to match x (32 KB)
    wv = np.ascontiguousarray(
        np.repeat(
            weight.reshape(COUT, CIN, KT).sum(axis=2).T.astype(np.float32), Q, axis=0
        )
    )
    rstd = (1.0 / np.sqrt(rv + np.float32(EPS))).astype(np.float32)
    alpha = (np.float32(SCALE / NSPATIAL) * rstd).astype(np.float32)
    beta = ((bias * np.float32(SCALE) - rm) * rstd).astype(np.float32)
    ab = np.ascontiguousarray(np.stack([alpha, beta], axis=1))
    return [
        {"x": xv[k * BPC : (k + 1) * BPC], "w": wv, "ab": ab}
        for k in range(NCORES)
    ]


def kernel(x, weight, bias, running_mean, running_var):
    global LAST_RESULT
    in_maps = prep_inputs(x, weight, bias, running_mean, running_var)
    nc = _build_program()
    res = run_bass_kernel_spmd(
        nc, in_maps, core_ids=list(range(NCORES)), trace=TRACE
    )
    LAST_RESULT = res

    out = np.empty((B, COUT), dtype=np.float32)
    for k in range(NCORES):
        out[k * BPC : (k + 1) * BPC] = res.results[k]["y"].T
    return out
